# revision 6
# baseline (speedup 1.0000x reference)
# MoE kernel for Trainium2 (8 NeuronCores, expert-parallel).
#
# Strategy:
#  - Host: gate logits = x @ gate_w, top-2 + softmax, gather tokens per expert
#    (the "all-to-all by routed expert" from the sharding hint, done host-side
#    since we hold full inputs), pad each expert's token set to a common
#    capacity C (= max expert load, rounded to 128).
#  - Device (core e = expert e): h = gelu(xg^T-major GEMM w1) ; y = h GEMM w2.
#    Both GEMMs in bf16 on the PE array (1 cycle/row), fp32 PSUM accumulate.
#    Token dim rides the matmul free axis; D/dff ride partitions.
#  - Host: scatter-add wts * (y + b2[e]) back into the output.
import math
import os
from contextlib import ExitStack

import ml_dtypes
import numpy as np

import concourse.bass as bass
import concourse.mybir as mybir
import concourse.tile as tile
from concourse.bass_utils import run_bass_kernel_spmd

D = 1024
DFF = 4096
E = 8
TOP_K = 2
P = 128
KD = D // P      # 8  contraction tiles for GEMM1
NF = DFF // P    # 32 dff tiles (GEMM1 out / GEMM2 contraction)
ND = D // P      # 8  GEMM2 out tiles
T_TILE = 512

BF16 = mybir.dt.bfloat16
F32 = mybir.dt.float32
NP_BF16 = np.dtype(ml_dtypes.bfloat16)

_neff_cache = {}


def _split_multiwait_json(bir_bytes: bytes) -> bytes:
    """The walrus build in this container rejects instructions carrying more
    than one sync wait (and likely update). Split extras onto adjacent
    single-wait EventSemaphore carriers on the same engine: program order on
    the engine preserves the semantics exactly."""
    import json as _json

    bir = _json.loads(bir_bytes)
    for fn in bir["functions"]:
        for blk in fn["blocks"]:
            insts = blk.get("instructions", [])
            out = []
            for inst in insts:
                si = inst.get("sync_info")
                if si:
                    waits = si.get("on_wait") or []
                    if len(waits) > 1:
                        for i, w in enumerate(waits[:-1]):
                            out.append({
                                "debug": inst.get("debug", 0),
                                "engine": inst["engine"],
                                "ins": [],
                                "name": f"{inst['name']}_w{i}",
                                "opcode": "EventSemaphore",
                                "outs": [],
                                "sync_info": {"on_update": [], "on_wait": [w]},
                            })
                        si["on_wait"] = [waits[-1]]
                out.append(inst)
                if si:
                    ups = si.get("on_update") or []
                    if len(ups) > 1:
                        for i, u in enumerate(ups[1:]):
                            out.append({
                                "debug": inst.get("debug", 0),
                                "engine": inst["engine"],
                                "ins": [],
                                "name": f"{inst['name']}_u{i}",
                                "opcode": "EventSemaphore",
                                "outs": [],
                                "sync_info": {"on_update": [u], "on_wait": []},
                            })
                        si["on_update"] = [ups[0]]
            blk["instructions"] = out
    return _json.dumps(bir).encode()


def _patch_to_json(nc: bass.Bass) -> bass.Bass:
    orig = nc.to_json_bytes
    nc.to_json_bytes = lambda: _split_multiwait_json(orig())
    return nc


def _build_bass(C: int) -> bass.Bass:
    """One expert's MLP: y[D, C] = gelu(xg[D, C]^T @ w1 + b1) @ w2  (layouts
    are transposed: token dim is the free axis everywhere)."""
    nc = bass.Bass()
    xg_h = nc.dram_tensor("xg", [D, C], BF16, kind="ExternalInput")
    w1_h = nc.dram_tensor("w1", [D, DFF], BF16, kind="ExternalInput")
    b1_h = nc.dram_tensor("b1", [DFF], F32, kind="ExternalInput")
    w2_h = nc.dram_tensor("w2", [DFF, D], BF16, kind="ExternalInput")
    y_h = nc.dram_tensor("y", [D, C], F32, kind="ExternalOutput")

    t_tiles = []
    t0 = 0
    while t0 < C:
        tsz = min(T_TILE, C - t0)
        t_tiles.append((t0, tsz))
        t0 += tsz

    with ExitStack() as ctx:
        tc = ctx.enter_context(tile.TileContext(nc))
        wpool = ctx.enter_context(tc.tile_pool(name="w", bufs=1))
        xpool = ctx.enter_context(tc.tile_pool(name="x", bufs=1))
        hpool = ctx.enter_context(tc.tile_pool(name="h", bufs=1))
        bpool = ctx.enter_context(tc.tile_pool(name="b", bufs=1))
        ypool = ctx.enter_context(tc.tile_pool(name="y", bufs=3))
        ps1 = ctx.enter_context(tc.tile_pool(name="ps1", bufs=3, space="PSUM"))
        ps2 = ctx.enter_context(tc.tile_pool(name="ps2", bufs=3, space="PSUM"))

        # Resident tensors, chunked so consumers wait on fine-grained DMAs.
        x_t = []
        for k in range(KD):
            t = xpool.tile([P, C], BF16, tag=f"x{k}", name=f"x{k}")
            nc.sync.dma_start(t[:], xg_h[k * P:(k + 1) * P, :])
            x_t.append(t)
        w1_t = []
        for k in range(KD):
            t = wpool.tile([P, DFF], BF16, tag=f"w1_{k}", name=f"w1_{k}")
            nc.sync.dma_start(t[:], w1_h[k * P:(k + 1) * P, :])
            w1_t.append(t)
        w2_t = []
        for f in range(NF):
            t = wpool.tile([P, D], BF16, tag=f"w2_{f}", name=f"w2_{f}")
            nc.sync.dma_start(t[:], w2_h[f * P:(f + 1) * P, :])
            w2_t.append(t)
        b1_raw = bpool.tile([P, NF], F32)
        nc.gpsimd.dma_start(b1_raw[:], b1_h[:].rearrange("(f p) -> p f", p=P))
        # Funnel b1 through an ACT-engine copy: downstream gelus then reach it
        # via same-engine program order, keeping each ACTIVATE at one sync wait
        # (the gen3 AC instruction encoding rejects two).
        b1_t = bpool.tile([P, NF], F32)
        nc.scalar.copy(b1_t[:], b1_raw[:])

        gelu = mybir.ActivationFunctionType.Gelu
        for (t0, tsz) in t_tiles:
            h_t = [hpool.tile([P, T_TILE], BF16, tag=f"h{f}", name=f"h{f}") for f in range(NF)]
            for f in range(NF):
                pt = ps1.tile([P, T_TILE], F32, tag="ps1", name="pt1")
                for k in range(KD):
                    nc.tensor.matmul(
                        pt[:, :tsz],
                        w1_t[k][:, f * P:(f + 1) * P],
                        x_t[k][:, t0:t0 + tsz],
                        start=(k == 0),
                        stop=(k == KD - 1),
                    )
                nc.scalar.activation(
                    h_t[f][:, :tsz], pt[:, :tsz], gelu, bias=b1_t[:, f:f + 1]
                )
            for dd in range(ND):
                pt2 = ps2.tile([P, T_TILE], F32, tag="ps2", name="pt2")
                for f in range(NF):
                    nc.tensor.matmul(
                        pt2[:, :tsz],
                        w2_t[f][:, dd * P:(dd + 1) * P],
                        h_t[f][:, :tsz],
                        start=(f == 0),
                        stop=(f == NF - 1),
                    )
                y_t = ypool.tile([P, T_TILE], F32, tag="y", name="yt")
                nc.vector.tensor_copy(y_t[:, :tsz], pt2[:, :tsz])
                nc.sync.dma_start(y_h[dd * P:(dd + 1) * P, t0:t0 + tsz], y_t[:, :tsz])
    return _patch_to_json(nc)


def _route(xf: np.ndarray, gate_w: np.ndarray):
    """Top-2 gating identical to the reference (argmax ties -> lower index)."""
    N = xf.shape[0]
    logits = xf @ gate_w  # (N, E) f32
    rows = np.arange(N)
    i1 = logits.argmax(1)
    v1 = logits[rows, i1]
    masked = logits.copy()
    masked[rows, i1] = -np.inf
    i2 = masked.argmax(1)
    v2 = masked[rows, i2]
    # softmax over the two selected logits (v1 >= v2)
    e = np.exp((v2 - v1).astype(np.float32))
    wt1 = (1.0 / (1.0 + e)).astype(np.float32)
    wt2 = (e / (1.0 + e)).astype(np.float32)
    idx_e, wts_e = [], []
    for ex in range(E):
        s1 = np.nonzero(i1 == ex)[0]
        s2 = np.nonzero(i2 == ex)[0]
        idx_e.append(np.concatenate([s1, s2]))
        wts_e.append(np.concatenate([wt1[s1], wt2[s2]]).astype(np.float32))
    return idx_e, wts_e


def kernel(x, gate_w, w1, b1, w2, b2, _trace=False):
    B, T, D_ = x.shape
    N = B * T
    xf = np.ascontiguousarray(x.reshape(N, D_).astype(np.float32))
    idx_e, wts_e = _route(xf, gate_w.astype(np.float32))
    cnts = [len(i) for i in idx_e]
    C = max(P, int(math.ceil(max(cnts) / P)) * P)

    if C in _neff_cache:
        nc = _neff_cache[C]
    else:
        nc = _build_bass(C)
        _neff_cache[C] = nc

    in_maps = []
    for ex in range(E):
        cnt = cnts[ex]
        xg = np.zeros((C, D), np.float32)
        if cnt:
            xg[:cnt] = xf[idx_e[ex]]
        in_maps.append({
            "xg": np.ascontiguousarray(xg.T).astype(NP_BF16),
            "w1": np.ascontiguousarray(w1[ex]).astype(NP_BF16),
            "b1": np.ascontiguousarray(b1[ex]).astype(np.float32),
            "w2": np.ascontiguousarray(w2[ex]).astype(NP_BF16),
        })

    res = run_bass_kernel_spmd(nc, in_maps, core_ids=list(range(E)), trace=_trace)
    if _trace:
        print(f"HW exec time: {res.exec_time_ns} ns")

    out = np.zeros((N, D), np.float32)
    for ex in range(E):
        cnt = cnts[ex]
        if not cnt:
            continue
        y = res.results[ex]["y"]  # (D, C) f32
        yt = y[:, :cnt].T + b2[ex][None, :].astype(np.float32)
        out[idx_e[ex]] += wts_e[ex][:, None] * yt
    return out.reshape(B, T, D_)


# revision 8
# speedup vs baseline: 1.0638x; 1.0638x over previous
# MoE kernel for Trainium2 (8 NeuronCores, expert-parallel).
#
# Strategy:
#  - Host: gate logits = x @ gate_w, top-2 + softmax, gather tokens per expert
#    (the "all-to-all by routed expert" from the sharding hint, done host-side
#    since we hold full inputs), pad each expert's token set to a common
#    capacity C (= max expert load, rounded to 128).
#  - Device (core e = expert e): h = gelu(xg^T-major GEMM w1) ; y = h GEMM w2.
#    Both GEMMs in bf16 on the PE array (1 cycle/row), fp32 PSUM accumulate.
#    Token dim rides the matmul free axis; D/dff ride partitions.
#  - Host: scatter-add wts * (y + b2[e]) back into the output.
import math
import os
from contextlib import ExitStack

import ml_dtypes
import numpy as np

import concourse.bass as bass
import concourse.mybir as mybir
import concourse.tile as tile
from concourse.bass_utils import run_bass_kernel_spmd

D = 1024
DFF = 4096
E = 8
TOP_K = 2
P = 128
KD = D // P      # 8  contraction tiles for GEMM1
NF = DFF // P    # 32 dff tiles (GEMM1 out / GEMM2 contraction)
ND = D // P      # 8  GEMM2 out tiles
T_TILE = 512

BF16 = mybir.dt.bfloat16
F32 = mybir.dt.float32
NP_BF16 = np.dtype(ml_dtypes.bfloat16)

_neff_cache = {}


def _split_multiwait_json(bir_bytes: bytes) -> bytes:
    """The walrus build in this container rejects instructions carrying more
    than one sync wait (and likely update). Split extras onto adjacent
    single-wait EventSemaphore carriers on the same engine: program order on
    the engine preserves the semantics exactly."""
    import json as _json

    bir = _json.loads(bir_bytes)
    for fn in bir["functions"]:
        for blk in fn["blocks"]:
            insts = blk.get("instructions", [])
            out = []
            for inst in insts:
                si = inst.get("sync_info")
                if si:
                    waits = si.get("on_wait") or []
                    if len(waits) > 1:
                        for i, w in enumerate(waits[:-1]):
                            out.append({
                                "debug": inst.get("debug", 0),
                                "engine": inst["engine"],
                                "ins": [],
                                "name": f"{inst['name']}_w{i}",
                                "opcode": "EventSemaphore",
                                "outs": [],
                                "sync_info": {"on_update": [], "on_wait": [w]},
                            })
                        si["on_wait"] = [waits[-1]]
                out.append(inst)
                if si:
                    ups = si.get("on_update") or []
                    if len(ups) > 1:
                        for i, u in enumerate(ups[1:]):
                            out.append({
                                "debug": inst.get("debug", 0),
                                "engine": inst["engine"],
                                "ins": [],
                                "name": f"{inst['name']}_u{i}",
                                "opcode": "EventSemaphore",
                                "outs": [],
                                "sync_info": {"on_update": [u], "on_wait": []},
                            })
                        si["on_update"] = [ups[0]]
            blk["instructions"] = out
    return _json.dumps(bir).encode()


def _patch_to_json(nc: bass.Bass) -> bass.Bass:
    orig = nc.to_json_bytes
    nc.to_json_bytes = lambda: _split_multiwait_json(orig())
    return nc


def _build_bass(C: int) -> bass.Bass:
    """One expert's MLP: y[D, C] = gelu(xg[D, C]^T @ w1 + b1) @ w2  (layouts
    are transposed: token dim is the free axis everywhere)."""
    nc = bass.Bass()
    xg_h = nc.dram_tensor("xg", [D, C], BF16, kind="ExternalInput")
    w1_h = nc.dram_tensor("w1", [D, DFF], BF16, kind="ExternalInput")
    b1_h = nc.dram_tensor("b1", [DFF], F32, kind="ExternalInput")
    w2_h = nc.dram_tensor("w2", [DFF, D], BF16, kind="ExternalInput")
    y_h = nc.dram_tensor("y", [D, C], F32, kind="ExternalOutput")

    t_tiles = []
    t0 = 0
    while t0 < C:
        tsz = min(T_TILE, C - t0)
        t_tiles.append((t0, tsz))
        t0 += tsz

    with ExitStack() as ctx:
        tc = ctx.enter_context(tile.TileContext(nc))
        wpool = ctx.enter_context(tc.tile_pool(name="w", bufs=1))
        xpool = ctx.enter_context(tc.tile_pool(name="x", bufs=1))
        hpool = ctx.enter_context(tc.tile_pool(name="h", bufs=1))
        bpool = ctx.enter_context(tc.tile_pool(name="b", bufs=1))
        ypool = ctx.enter_context(tc.tile_pool(name="y", bufs=3))
        ps1 = ctx.enter_context(tc.tile_pool(name="ps1", bufs=3, space="PSUM"))
        ps2 = ctx.enter_context(tc.tile_pool(name="ps2", bufs=3, space="PSUM"))

        # Resident tensors, chunked so consumers wait on fine-grained DMAs and
        # ordered so the first GEMM1 f-group's data (xg t-tile 0 + w1 f=0)
        # lands first: PE starts ~3.5us in instead of ~22us.
        x_t = [[None] * len(t_tiles) for _ in range(KD)]
        for ti, (t0, tsz) in enumerate(t_tiles):
            for k in range(KD):
                t = xpool.tile([P, T_TILE], BF16, tag=f"x{k}_{ti}", name=f"x{k}_{ti}")
                nc.sync.dma_start(t[:, :tsz], xg_h[k * P:(k + 1) * P, t0:t0 + tsz])
                x_t[k][ti] = t
            if ti == 0:
                w1_t = []
                for f in range(NF):
                    t = wpool.tile([P, KD, P], BF16, tag=f"w1_{f}", name=f"w1_{f}")
                    nc.sync.dma_start(
                        t[:],
                        w1_h[:, f * P:(f + 1) * P].rearrange("(kd p) m -> p kd m", p=P),
                    )
                    w1_t.append(t)
        w2_t = []
        for f in range(NF):
            t = wpool.tile([P, D], BF16, tag=f"w2_{f}", name=f"w2_{f}")
            nc.sync.dma_start(t[:], w2_h[f * P:(f + 1) * P, :])
            w2_t.append(t)
        b1_raw = bpool.tile([P, NF], F32)
        nc.gpsimd.dma_start(b1_raw[:], b1_h[:].rearrange("(f p) -> p f", p=P))
        # Funnel b1 through an ACT-engine copy: downstream gelus then reach it
        # via same-engine program order, keeping each ACTIVATE at one sync wait
        # (the gen3 AC instruction encoding rejects two).
        b1_t = bpool.tile([P, NF], F32)
        nc.scalar.copy(b1_t[:], b1_raw[:])

        gelu = mybir.ActivationFunctionType.Gelu
        for ti, (t0, tsz) in enumerate(t_tiles):
            h_t = [hpool.tile([P, T_TILE], BF16, tag=f"h{f}", name=f"h{f}") for f in range(NF)]
            for f in range(NF):
                pt = ps1.tile([P, T_TILE], F32, tag="ps1", name="pt1")
                for k in range(KD):
                    nc.tensor.matmul(
                        pt[:, :tsz],
                        w1_t[f][:, k, :],
                        x_t[k][ti][:, :tsz],
                        start=(k == 0),
                        stop=(k == KD - 1),
                    )
                nc.scalar.activation(
                    h_t[f][:, :tsz], pt[:, :tsz], gelu, bias=b1_t[:, f:f + 1]
                )
            for dd in range(ND):
                pt2 = ps2.tile([P, T_TILE], F32, tag="ps2", name="pt2")
                for f in range(NF):
                    nc.tensor.matmul(
                        pt2[:, :tsz],
                        w2_t[f][:, dd * P:(dd + 1) * P],
                        h_t[f][:, :tsz],
                        start=(f == 0),
                        stop=(f == NF - 1),
                    )
                y_t = ypool.tile([P, T_TILE], F32, tag="y", name="yt")
                nc.vector.tensor_copy(y_t[:, :tsz], pt2[:, :tsz])
                nc.sync.dma_start(y_h[dd * P:(dd + 1) * P, t0:t0 + tsz], y_t[:, :tsz])
    return _patch_to_json(nc)


def _route(xf: np.ndarray, gate_w: np.ndarray):
    """Top-2 gating identical to the reference (argmax ties -> lower index)."""
    N = xf.shape[0]
    logits = xf @ gate_w  # (N, E) f32
    rows = np.arange(N)
    i1 = logits.argmax(1)
    v1 = logits[rows, i1]
    masked = logits.copy()
    masked[rows, i1] = -np.inf
    i2 = masked.argmax(1)
    v2 = masked[rows, i2]
    # softmax over the two selected logits (v1 >= v2)
    e = np.exp((v2 - v1).astype(np.float32))
    wt1 = (1.0 / (1.0 + e)).astype(np.float32)
    wt2 = (e / (1.0 + e)).astype(np.float32)
    idx_e, wts_e = [], []
    for ex in range(E):
        s1 = np.nonzero(i1 == ex)[0]
        s2 = np.nonzero(i2 == ex)[0]
        idx_e.append(np.concatenate([s1, s2]))
        wts_e.append(np.concatenate([wt1[s1], wt2[s2]]).astype(np.float32))
    return idx_e, wts_e


def kernel(x, gate_w, w1, b1, w2, b2, _trace=False):
    B, T, D_ = x.shape
    N = B * T
    xf = np.ascontiguousarray(x.reshape(N, D_).astype(np.float32))
    idx_e, wts_e = _route(xf, gate_w.astype(np.float32))
    cnts = [len(i) for i in idx_e]
    C = max(P, int(math.ceil(max(cnts) / P)) * P)

    if C in _neff_cache:
        nc = _neff_cache[C]
    else:
        nc = _build_bass(C)
        _neff_cache[C] = nc

    in_maps = []
    for ex in range(E):
        cnt = cnts[ex]
        xg = np.zeros((C, D), np.float32)
        if cnt:
            xg[:cnt] = xf[idx_e[ex]]
        in_maps.append({
            "xg": np.ascontiguousarray(xg.T).astype(NP_BF16),
            "w1": np.ascontiguousarray(w1[ex]).astype(NP_BF16),
            "b1": np.ascontiguousarray(b1[ex]).astype(np.float32),
            "w2": np.ascontiguousarray(w2[ex]).astype(NP_BF16),
        })

    res = run_bass_kernel_spmd(nc, in_maps, core_ids=list(range(E)), trace=_trace)
    if _trace:
        print(f"HW exec time: {res.exec_time_ns} ns")

    out = np.zeros((N, D), np.float32)
    for ex in range(E):
        cnt = cnts[ex]
        if not cnt:
            continue
        y = res.results[ex]["y"]  # (D, C) f32
        yt = y[:, :cnt].T + b2[ex][None, :].astype(np.float32)
        out[idx_e[ex]] += wts_e[ex][:, None] * yt
    return out.reshape(B, T, D_)
